# revision 1
# baseline (speedup 1.0000x reference)
"""CausalSelfAttention (depthwise-conv + RoPE + causal SDPA + proj) on 8 Trainium2 cores.

Tensor-parallel over heads: each core computes 2 of 16 heads end-to-end plus its
partial output projection; the host sums the 8 partial projections.
All matmuls run in float32r. Layouts are transposed ([dim, time]) so DMA is
contiguous and softmax denominators come from a PE ones-matmul."""
import sys
sys.path.insert(0, '/opt/trn_rl_repo')
import numpy as np
import concourse.bass as bass
import concourse.mybir as mybir
import concourse.tile as tile
from concourse import bacc
from concourse import bass_utils

F32 = mybir.dt.float32
F32R = mybir.dt.float32r
AF = mybir.ActivationFunctionType
OP = mybir.AluOpType

B, T, C = 2, 2048, 2048
H, D = 16, 128
NC = 8
HPC = H // NC          # heads per core = 2
CHW = 512              # chunk width (t)
NCH = T // CHW         # chunks per batch = 4
CT = C // 128          # 16 c-tiles
STW = 128              # s-tile width


def build_program():
    nc = bacc.Bacc("TRN2", target_bir_lowering=False, debug=False, num_devices=NC)

    xT = nc.dram_tensor("xT", [C, B * T], F32, kind="ExternalInput").ap()
    wq = nc.dram_tensor("wq", [C, HPC * D], F32, kind="ExternalInput").ap()
    wk = nc.dram_tensor("wk", [C, HPC * D], F32, kind="ExternalInput").ap()
    wv = nc.dram_tensor("wv", [C, HPC * D], F32, kind="ExternalInput").ap()
    wo = nc.dram_tensor("wo", [HPC * D, C], F32, kind="ExternalInput").ap()
    cosT_d = nc.dram_tensor("cosT", [D, T], F32, kind="ExternalInput").ap()
    sinS_d = nc.dram_tensor("sinS", [D, T], F32, kind="ExternalInput").ap()
    masks_d = nc.dram_tensor("masks", [STW, 4 * CHW], F32, kind="ExternalInput").ap()
    convw_d = nc.dram_tensor("convw", [C, 3], F32, kind="ExternalInput").ap()
    ones_d = nc.dram_tensor("ones", [128, 128], F32, kind="ExternalInput").ap()
    ident_d = nc.dram_tensor("ident", [128, 128], F32, kind="ExternalInput").ap()
    outT = nc.dram_tensor("outT", [C, B * T], F32, kind="ExternalOutput").ap()

    with tile.TileContext(nc) as tc:
        with (
            tc.tile_pool(name="wr", bufs=1) as wr,
            tc.tile_pool(name="const", bufs=1) as cst,
            tc.tile_pool(name="wstg", bufs=4) as wstg,
            tc.tile_pool(name="xt", bufs=2) as xtp,
            tc.tile_pool(name="xc", bufs=1) as xcp,
            tc.tile_pool(name="tmp", bufs=2) as tmp,
            tc.tile_pool(name="rd", bufs=2) as rdp,
            tc.tile_pool(name="qs", bufs=1) as qsp,
            tc.tile_pool(name="kv", bufs=1) as kvp,
            tc.tile_pool(name="vstg", bufs=2) as vstgp,
            tc.tile_pool(name="e", bufs=3) as ep,
            tc.tile_pool(name="y", bufs=1) as yp,
            tc.tile_pool(name="pmm", bufs=2, space="PSUM") as pmm,
            tc.tile_pool(name="pS", bufs=2, space="PSUM") as pS,
            tc.tile_pool(name="pU", bufs=2, space="PSUM") as pU,
            tc.tile_pool(name="pD", bufs=2, space="PSUM") as pD,
        ):
            # ---- constants ----
            cosT = cst.tile([D, T], F32, tag="cosT")
            nc.sync.dma_start(cosT[:], cosT_d[:])
            sinS = cst.tile([D, T], F32, tag="sinS")
            nc.sync.dma_start(sinS[:], sinS_d[:])
            masks = cst.tile([STW, 4 * CHW], F32, tag="masks")
            nc.sync.dma_start(masks[:], masks_d[:])
            ident = cst.tile([128, 128], F32, tag="ident")
            nc.sync.dma_start(ident[:], ident_d[:])
            cw = []
            for ct in range(CT):
                t_ = cst.tile([128, 3], F32, tag=f"cw{ct}")
                nc.sync.dma_start(t_[:], convw_d[ct * 128:(ct + 1) * 128, :])
                cw.append(t_)

            ones_st = wstg.tile([128, 128], F32, tag="wst_on")
            nc.sync.dma_start(ones_st[:], ones_d[:])
            ones_r = wr.tile([128, 128], F32R, tag="ones_r")
            nc.vector.tensor_copy(ones_r[:], ones_st[:])

            # ---- weights -> f32r resident tiles ----
            wq_r, wk_r, wv_r = [], [], []
            for name, dsrc, dst in (("q", wq, wq_r), ("k", wk, wk_r), ("v", wv, wv_r)):
                for ct in range(CT):
                    stg = wstg.tile([128, HPC * D], F32, tag="wst")
                    nc.sync.dma_start(stg[:], dsrc[ct * 128:(ct + 1) * 128, :])
                    t_ = wr.tile([128, HPC * D], F32R, tag=f"w{name}r{ct}")
                    nc.vector.tensor_copy(t_[:], stg[:])
                    dst.append(t_)
            wo_r = []
            for hi in range(HPC):
                t_ = wr.tile([128, C], F32R, tag=f"wor{hi}")
                for j in range(C // 256):
                    stg = wstg.tile([128, 256], F32, tag="wst_o")
                    nc.sync.dma_start(stg[:], wo[hi * 128:(hi + 1) * 128, j * 256:(j + 1) * 256])
                    nc.vector.tensor_copy(t_[:, j * 256:(j + 1) * 256], stg[:])
                wo_r.append(t_)

            # ---- main loop ----
            for b in range(B):
                k_all = [kvp.tile([D, T], F32R, tag=f"k{h}", name=f"kall{b}_{h}") for h in range(HPC)]
                v_all = [kvp.tile([128, T], F32R, tag=f"v{h}", name=f"vall{b}_{h}") for h in range(HPC)]
                for ch in range(NCH):
                    g0 = b * T + ch * CHW          # global col offset into xT/outT
                    t0 = ch * CHW                  # within-batch t offset
                    # ---- conv: xc[ct] = depthwise causal conv ----
                    xc = []
                    for ct in range(CT):
                        xt = xtp.tile([128, CHW + 2], F32, tag="xt")
                        if ch == 0:
                            nc.gpsimd.memset(xt[:, 0:2], 0.0)
                            nc.sync.dma_start(xt[:, 2:CHW + 2], xT[ct * 128:(ct + 1) * 128, g0:g0 + CHW])
                        else:
                            nc.sync.dma_start(xt[:], xT[ct * 128:(ct + 1) * 128, g0 - 2:g0 + CHW])
                        ta = tmp.tile([128, CHW], F32, tag="t1")
                        nc.scalar.mul(ta[:], xt[:, 0:CHW], cw[ct][:, 0:1])
                        tb = tmp.tile([128, CHW], F32, tag="t2")
                        nc.vector.scalar_tensor_tensor(tb[:], xt[:, 1:CHW + 1], cw[ct][:, 1:2], ta[:], OP.mult, OP.add)
                        xct = xcp.tile([128, CHW], F32R, tag=f"xc{ct}")
                        nc.vector.scalar_tensor_tensor(xct[:], xt[:, 2:CHW + 2], cw[ct][:, 2:3], tb[:], OP.mult, OP.add)
                        xc.append(xct)

                    # ---- QKV + rope ----
                    q_sb = []
                    for h in range(HPC):
                        hs = slice(h * D, (h + 1) * D)
                        # q
                        q_ps = pmm.tile([128, CHW], F32, tag="mm")
                        for ct in range(CT):
                            nc.tensor.matmul(q_ps[:], wq_r[ct][:, hs], xc[ct][:],
                                             start=(ct == 0), stop=(ct == CT - 1))
                        a = tmp.tile([128, CHW], F32, tag="t1")
                        nc.vector.tensor_tensor(a[:], q_ps[:], cosT[:, t0:t0 + CHW], OP.mult)
                        bb = tmp.tile([128, CHW], F32, tag="t2")
                        nc.vector.tensor_tensor(bb[0:64, :], q_ps[64:128, :], sinS[0:64, t0:t0 + CHW], OP.mult)
                        nc.vector.tensor_tensor(bb[64:128, :], q_ps[0:64, :], sinS[64:128, t0:t0 + CHW], OP.mult)
                        qt = qsp.tile([128, CHW], F32R, tag=f"q{h}")
                        nc.vector.tensor_tensor(qt[:], a[:], bb[:], OP.add)
                        q_sb.append(qt)
                        # k
                        k_ps = pmm.tile([128, CHW], F32, tag="mm")
                        for ct in range(CT):
                            nc.tensor.matmul(k_ps[:], wk_r[ct][:, hs], xc[ct][:],
                                             start=(ct == 0), stop=(ct == CT - 1))
                        a2 = tmp.tile([128, CHW], F32, tag="t1")
                        nc.vector.tensor_tensor(a2[:], k_ps[:], cosT[:, t0:t0 + CHW], OP.mult)
                        b2 = tmp.tile([128, CHW], F32, tag="t2")
                        nc.vector.tensor_tensor(b2[0:64, :], k_ps[64:128, :], sinS[0:64, t0:t0 + CHW], OP.mult)
                        nc.vector.tensor_tensor(b2[64:128, :], k_ps[0:64, :], sinS[64:128, t0:t0 + CHW], OP.mult)
                        nc.vector.tensor_tensor(k_all[h][:, t0:t0 + CHW], a2[:], b2[:], OP.add)
                        # v
                        v_ps = pmm.tile([128, CHW], F32, tag="mm")
                        for ct in range(CT):
                            nc.tensor.matmul(v_ps[:], wv_r[ct][:, hs], xc[ct][:],
                                             start=(ct == 0), stop=(ct == CT - 1))
                        vstg = vstgp.tile([128, CHW], F32, tag="vstg")
                        nc.scalar.copy(vstg[:], v_ps[:])
                        for j in range(CHW // 128):
                            tp = pS.tile([128, 128], F32, tag="S")
                            nc.tensor.transpose(tp[:], vstg[:, j * 128:(j + 1) * 128], ident[:])
                            srow = t0 + j * 128
                            nc.vector.tensor_copy(v_all[h][:, srow:srow + 128], tp[:])

                    # ---- attention ----
                    yT = []
                    n_st = 4 * ch + 4
                    for h in range(HPC):
                        U_ps = pU.tile([128, CHW], F32, tag="U")
                        D_ps = pD.tile([128, CHW], F32, tag="Dn")
                        for st in range(n_st):
                            s_ps = pS.tile([128, CHW], F32, tag="S")
                            nc.tensor.matmul(s_ps[:], k_all[h][:, st * STW:(st + 1) * STW], q_sb[h][:],
                                             start=True, stop=True)
                            e = ep.tile([128, CHW], F32R, tag="e")
                            nc.scalar.activation(e[:], s_ps[:], AF.Exp)
                            if st >= 4 * ch:
                                i = st - 4 * ch
                                nc.vector.tensor_tensor(e[:], e[:], masks[:, i * CHW:(i + 1) * CHW], OP.mult)
                            nc.tensor.matmul(U_ps[:], v_all[h][:, st * STW:(st + 1) * STW], e[:],
                                             start=(st == 0), stop=(st == n_st - 1))
                            nc.tensor.matmul(D_ps[:], ones_r[:], e[:],
                                             start=(st == 0), stop=(st == n_st - 1))
                        rD = rdp.tile([128, CHW], F32, tag="rd")
                        nc.vector.reciprocal(rD[:], D_ps[:])
                        yt = yp.tile([128, CHW], F32R, tag=f"y{h}")
                        nc.vector.tensor_tensor(yt[:], U_ps[:], rD[:], OP.mult)
                        yT.append(yt)

                    # ---- proj (partial over this core's heads) ----
                    for oc in range(CT):
                        o_ps = pmm.tile([128, CHW], F32, tag="mm")
                        nc.tensor.matmul(o_ps[:], wo_r[0][:, oc * 128:(oc + 1) * 128], yT[0][:],
                                         start=True, stop=False)
                        nc.tensor.matmul(o_ps[:], wo_r[1][:, oc * 128:(oc + 1) * 128], yT[1][:],
                                         start=False, stop=True)
                        o_sb = vstgp.tile([128, CHW], F32, tag="osb")
                        nc.scalar.copy(o_sb[:], o_ps[:])
                        nc.sync.dma_start(outT[oc * 128:(oc + 1) * 128, g0:g0 + CHW], o_sb[:])

    nc.compile()
    return nc


def host_prepare(x, conv_w, w_attn, w_proj):
    """Build per-core input maps."""
    xT = np.ascontiguousarray(x.transpose(2, 0, 1).reshape(C, B * T)).astype(np.float32)
    convw = np.ascontiguousarray(conv_w[:, 0, :]).astype(np.float32)

    t = np.arange(T, dtype=np.float64)
    inv_freq = 1.0 / (10000.0 ** (np.arange(0, D, 2, dtype=np.float64) / D))
    freqs = np.outer(t, inv_freq)                      # [T, 64]
    emb = np.concatenate([freqs, freqs], axis=1)       # [T, 128]
    cosT = np.cos(emb).T.astype(np.float32)            # [128, T]
    sinT = np.sin(emb).T.astype(np.float32)
    sinS = sinT.copy()
    sinS[0:64] = -sinT[0:64]

    masks = np.zeros((STW, 4 * CHW), dtype=np.float32)
    s_idx = np.arange(STW)[:, None]
    t_idx = np.arange(CHW)[None, :]
    for i in range(4):
        masks[:, i * CHW:(i + 1) * CHW] = (i * STW + s_idx <= t_idx).astype(np.float32)

    ones = np.ones((128, 128), dtype=np.float32)
    ident = np.eye(128, dtype=np.float32)

    scale = 1.0 / np.sqrt(np.float32(D))
    in_maps = []
    for c in range(NC):
        h0 = c * HPC
        rq = slice(h0 * D, (h0 + HPC) * D)
        wq_c = np.ascontiguousarray((w_attn[rq.start:rq.stop, :] * scale).T).astype(np.float32)
        wk_c = np.ascontiguousarray(w_attn[C + rq.start:C + rq.stop, :].T).astype(np.float32)
        wv_c = np.ascontiguousarray(w_attn[2 * C + rq.start:2 * C + rq.stop, :].T).astype(np.float32)
        wo_c = np.ascontiguousarray(w_proj[:, rq.start:rq.stop].T).astype(np.float32)
        in_maps.append({
            "xT": xT, "wq": wq_c, "wk": wk_c, "wv": wv_c, "wo": wo_c,
            "cosT": cosT, "sinS": sinS, "masks": masks, "convw": convw,
            "ones": ones, "ident": ident,
        })
    return in_maps


def host_finish(results):
    acc = np.zeros((C, B * T), dtype=np.float64)
    for r in results:
        acc += r["outT"]
    return acc.reshape(C, B, T).transpose(1, 2, 0).astype(np.float32)


_CACHE = {}




def kernel(x, conv_w, w_attn, w_proj):
    x = np.ascontiguousarray(x, dtype=np.float32)
    conv_w = np.ascontiguousarray(conv_w, dtype=np.float32)
    w_attn = np.ascontiguousarray(w_attn, dtype=np.float32)
    w_proj = np.ascontiguousarray(w_proj, dtype=np.float32)
    if "nc" not in _CACHE:
        _CACHE["nc"] = build_program()
    in_maps = host_prepare(x, conv_w, w_attn, w_proj)
    res = bass_utils.run_bass_kernel_spmd(_CACHE["nc"], in_maps, core_ids=list(range(NC)))
    return host_finish(res.results)



# revision 8
# speedup vs baseline: 1.4326x; 1.4326x over previous
"""CausalSelfAttention (depthwise-conv + RoPE + causal SDPA + proj) on 8 Trainium2 cores.

v2: minimize per-call host<->device traffic (the dominant cost through this
dispatch path) and run compute in fp16 with fp32 accumulation.

- x is sequence-sharded: each core ships only its 512-token chunk (fp16,
  transposed, with a 2-token conv halo); a device-side AllGather rebuilds the
  full sequence on every core.
- Compute is tensor-parallel over heads (2 of 16 per core): conv -> QKV ->
  RoPE -> causal SDPA -> partial output projection, all matmuls fp16 in /
  fp32 PSUM accumulate.
- The partial projections are ReduceScattered per 512-token chunk, so each
  core returns only a [256, 4096] fp16 slice of the transposed output."""
import sys
sys.path.insert(0, '/opt/trn_rl_repo')
import numpy as np
import concourse.bass as bass
import concourse.mybir as mybir
import concourse.tile as tile
from concourse import bacc
from concourse import bass_utils

F32 = mybir.dt.float32
F16 = mybir.dt.float16
AF = mybir.ActivationFunctionType
OP = mybir.AluOpType

B, T, C = 2, 2048, 2048
H, D = 16, 128
NC = 8
HPC = H // NC          # heads per core = 2
CHW = 512              # chunk width (tokens)
NCH = T // CHW         # chunks per batch = 4
TCH = B * NCH          # total chunks = 8
CT = C // 128          # 16 c-tiles
STW = 128              # s-tile width
XW = CHW + 2           # shipped chunk width incl. 2-col conv halo


def build_program():
    nc = bacc.Bacc("TRN2", target_bir_lowering=False, debug=False, num_devices=NC)

    xsh_d = nc.dram_tensor("xsh", [C, XW], F16, kind="ExternalInput").ap()
    wq_d = nc.dram_tensor("wq", [C, HPC * D], F16, kind="ExternalInput").ap()
    wk_d = nc.dram_tensor("wk", [C, HPC * D], F16, kind="ExternalInput").ap()
    wv_d = nc.dram_tensor("wv", [C, HPC * D], F16, kind="ExternalInput").ap()
    wo_d = nc.dram_tensor("wo", [HPC * D, C], F16, kind="ExternalInput").ap()
    cos_d = nc.dram_tensor("cos64", [64, T], F32, kind="ExternalInput").ap()
    sin_d = nc.dram_tensor("sin64", [64, T], F32, kind="ExternalInput").ap()
    masks_d = nc.dram_tensor("masks", [STW, 4 * CHW], F16, kind="ExternalInput").ap()
    convw_d = nc.dram_tensor("convw", [C, 3], F32, kind="ExternalInput").ap()
    ones_d = nc.dram_tensor("ones", [128, 128], F16, kind="ExternalInput").ap()
    ident_d = nc.dram_tensor("ident", [128, 128], F16, kind="ExternalInput").ap()
    outp_d = nc.dram_tensor("outp", [2 * D, B * T], F16, kind="ExternalOutput").ap()

    from contextlib import ExitStack
    with tile.TileContext(nc) as tc:
        with ExitStack() as stack:
            dram = stack.enter_context(tc.tile_pool(name="dram", bufs=1, space="DRAM"))
            drp = stack.enter_context(tc.tile_pool(name="drp", bufs=2, space="DRAM"))
            wr = stack.enter_context(tc.tile_pool(name="wr", bufs=1))
            cst = stack.enter_context(tc.tile_pool(name="const", bufs=1))
            xtp = stack.enter_context(tc.tile_pool(name="xt", bufs=2))
            xcp = stack.enter_context(tc.tile_pool(name="xc", bufs=1))
            tmp = stack.enter_context(tc.tile_pool(name="tmp", bufs=2))
            rpp = stack.enter_context(tc.tile_pool(name="rp", bufs=2))
            rdp = stack.enter_context(tc.tile_pool(name="rd", bufs=2))
            qsp = stack.enter_context(tc.tile_pool(name="qs", bufs=1))
            kvp = stack.enter_context(tc.tile_pool(name="kv", bufs=1))
            vstgp = stack.enter_context(tc.tile_pool(name="vstg", bufs=2))
            ep = stack.enter_context(tc.tile_pool(name="e", bufs=3))
            yp = stack.enter_context(tc.tile_pool(name="y", bufs=1))
            obp = stack.enter_context(tc.tile_pool(name="ob", bufs=4))
            pmm = stack.enter_context(tc.tile_pool(name="pmm", bufs=2, space="PSUM"))
            pS = stack.enter_context(tc.tile_pool(name="pS", bufs=2, space="PSUM"))
            pU = stack.enter_context(tc.tile_pool(name="pU", bufs=2, space="PSUM"))
            pD = stack.enter_context(tc.tile_pool(name="pD", bufs=2, space="PSUM"))
            # ---- AllGather x shards: every core gets the full sequence ----
            ag_in = dram.tile([C, XW], F16, name="ag_in")
            nc.gpsimd.dma_start(ag_in[:], xsh_d[:])
            xg = dram.tile([NC * C, XW], F16, name="xg")
            nc.gpsimd.collective_compute(
                "AllGather", mybir.AluOpType.bypass,
                replica_groups=[list(range(NC))],
                ins=[ag_in.opt()], outs=[xg.opt()],
            )

            # ---- constants ----
            cos64 = cst.tile([64, T], F32, tag="cos64")
            nc.sync.dma_start(cos64[:], cos_d[:])
            sin64 = cst.tile([64, T], F32, tag="sin64")
            nc.sync.dma_start(sin64[:], sin_d[:])
            masks = cst.tile([STW, 4 * CHW], F16, tag="masks")
            nc.sync.dma_start(masks[:], masks_d[:])
            ident = cst.tile([128, 128], F16, tag="ident")
            nc.sync.dma_start(ident[:], ident_d[:])
            ones16 = cst.tile([128, 128], F16, tag="ones16")
            nc.sync.dma_start(ones16[:], ones_d[:])
            cw = []
            for ct in range(CT):
                t_ = cst.tile([128, 3], F32, tag=f"cw{ct}")
                nc.sync.dma_start(t_[:], convw_d[ct * 128:(ct + 1) * 128, :])
                cw.append(t_)

            # ---- weights resident in SBUF (fp16, used directly by PE) ----
            wq_r, wk_r, wv_r = [], [], []
            for name, dsrc, dst in (("q", wq_d, wq_r), ("k", wk_d, wk_r), ("v", wv_d, wv_r)):
                for ct in range(CT):
                    t_ = wr.tile([128, HPC * D], F16, tag=f"w{name}r{ct}")
                    nc.sync.dma_start(t_[:], dsrc[ct * 128:(ct + 1) * 128, :])
                    dst.append(t_)
            wo_r = []
            for hi in range(HPC):
                t_ = wr.tile([128, C], F16, tag=f"wor{hi}")
                nc.sync.dma_start(t_[:], wo_d[hi * 128:(hi + 1) * 128, :])
                wo_r.append(t_)

            # ---- main loop over the 8 chunks (b major, ch minor) ----
            for b in range(B):
                k_all = [kvp.tile([D, T], F16, tag=f"k{h}", name=f"kall{b}_{h}") for h in range(HPC)]
                v_all = [kvp.tile([128, T], F16, tag=f"v{h}", name=f"vall{b}_{h}") for h in range(HPC)]
                for ch in range(NCH):
                    g = b * NCH + ch               # global chunk id / xg block
                    t0 = ch * CHW                  # within-batch t offset
                    # ---- load + depthwise causal conv ----
                    xc = []
                    for ct in range(CT):
                        xt = xtp.tile([128, XW], F16, tag="xt")
                        nc.sync.dma_start(xt[:], xg[g * C + ct * 128:g * C + (ct + 1) * 128, :])
                        ta = tmp.tile([128, CHW], F16, tag="t1")
                        nc.scalar.mul(ta[:], xt[:, 0:CHW], cw[ct][:, 0:1])
                        tb = tmp.tile([128, CHW], F16, tag="t2")
                        nc.vector.scalar_tensor_tensor(tb[:], xt[:, 1:CHW + 1], cw[ct][:, 1:2], ta[:], OP.mult, OP.add)
                        xct = xcp.tile([128, CHW], F16, tag=f"xc{ct}")
                        nc.vector.scalar_tensor_tensor(xct[:], xt[:, 2:CHW + 2], cw[ct][:, 2:3], tb[:], OP.mult, OP.add)
                        xc.append(xct)

                    # ---- QKV + rope ----
                    q_sb = []
                    for h in range(HPC):
                        hs = slice(h * D, (h + 1) * D)
                        cs = slice(t0, t0 + CHW)
                        # q
                        q_ps = pmm.tile([128, CHW], F32, tag="mm")
                        for ct in range(CT):
                            nc.tensor.matmul(q_ps[:], wq_r[ct][:, hs], xc[ct][:],
                                             start=(ct == 0), stop=(ct == CT - 1))
                        qt = qsp.tile([128, CHW], F16, tag=f"q{h}")
                        at = rpp.tile([64, CHW], F16, tag="ra")
                        nc.vector.tensor_tensor(at[:], q_ps[0:64, :], cos64[:, cs], OP.mult)
                        mt = rpp.tile([64, CHW], F16, tag="rm")
                        nc.vector.tensor_tensor(mt[:], q_ps[64:128, :], sin64[:, cs], OP.mult)
                        nc.vector.tensor_tensor(qt[0:64, :], at[:], mt[:], OP.subtract)
                        ab = rpp.tile([64, CHW], F16, tag="rb")
                        nc.vector.tensor_tensor(ab[:], q_ps[64:128, :], cos64[:, cs], OP.mult)
                        mb = rpp.tile([64, CHW], F16, tag="rn")
                        nc.vector.tensor_tensor(mb[:], q_ps[0:64, :], sin64[:, cs], OP.mult)
                        nc.vector.tensor_tensor(qt[64:128, :], ab[:], mb[:], OP.add)
                        q_sb.append(qt)
                        # k
                        k_ps = pmm.tile([128, CHW], F32, tag="mm")
                        for ct in range(CT):
                            nc.tensor.matmul(k_ps[:], wk_r[ct][:, hs], xc[ct][:],
                                             start=(ct == 0), stop=(ct == CT - 1))
                        at2 = rpp.tile([64, CHW], F16, tag="ra")
                        nc.vector.tensor_tensor(at2[:], k_ps[0:64, :], cos64[:, cs], OP.mult)
                        mt2 = rpp.tile([64, CHW], F16, tag="rm")
                        nc.vector.tensor_tensor(mt2[:], k_ps[64:128, :], sin64[:, cs], OP.mult)
                        nc.vector.tensor_tensor(k_all[h][0:64, cs], at2[:], mt2[:], OP.subtract)
                        ab2 = rpp.tile([64, CHW], F16, tag="rb")
                        nc.vector.tensor_tensor(ab2[:], k_ps[64:128, :], cos64[:, cs], OP.mult)
                        mb2 = rpp.tile([64, CHW], F16, tag="rn")
                        nc.vector.tensor_tensor(mb2[:], k_ps[0:64, :], sin64[:, cs], OP.mult)
                        nc.vector.tensor_tensor(k_all[h][64:128, cs], ab2[:], mb2[:], OP.add)
                        # v
                        v_ps = pmm.tile([128, CHW], F32, tag="mm")
                        for ct in range(CT):
                            nc.tensor.matmul(v_ps[:], wv_r[ct][:, hs], xc[ct][:],
                                             start=(ct == 0), stop=(ct == CT - 1))
                        vstg = vstgp.tile([128, CHW], F16, tag="vstg")
                        nc.scalar.copy(vstg[:], v_ps[:])
                        for j in range(CHW // 128):
                            tp = pS.tile([128, 128], F16, tag="S")
                            nc.tensor.transpose(tp[:], vstg[:, j * 128:(j + 1) * 128], ident[:])
                            srow = t0 + j * 128
                            nc.vector.tensor_copy(v_all[h][:, srow:srow + 128], tp[:])

                    # ---- attention ----
                    yT = []
                    n_st = 4 * ch + 4
                    for h in range(HPC):
                        U_ps = pU.tile([128, CHW], F32, tag="U")
                        D_ps = pD.tile([128, CHW], F32, tag="Dn")
                        for st in range(n_st):
                            s_ps = pS.tile([128, CHW], F32, tag="S")
                            nc.tensor.matmul(s_ps[:], k_all[h][:, st * STW:(st + 1) * STW], q_sb[h][:],
                                             start=True, stop=True)
                            e = ep.tile([128, CHW], F16, tag="e")
                            nc.scalar.activation(e[:], s_ps[:], AF.Exp)
                            if st >= 4 * ch:
                                i = st - 4 * ch
                                nc.vector.tensor_tensor(e[:], e[:], masks[:, i * CHW:(i + 1) * CHW], OP.mult)
                            nc.tensor.matmul(U_ps[:], v_all[h][:, st * STW:(st + 1) * STW], e[:],
                                             start=(st == 0), stop=(st == n_st - 1))
                            nc.tensor.matmul(D_ps[:], ones16[:], e[:],
                                             start=(st == 0), stop=(st == n_st - 1))
                        rD = rdp.tile([128, CHW], F32, tag="rd")
                        nc.vector.reciprocal(rD[:], D_ps[:])
                        yt = yp.tile([128, CHW], F16, tag=f"y{h}")
                        nc.vector.tensor_tensor(yt[:], U_ps[:], rD[:], OP.mult)
                        yT.append(yt)

                    # ---- partial proj for this chunk -> DRAM -> ReduceScatter ----
                    partial = drp.tile([C, CHW], F16, tag="part", name=f"part{g}")
                    for oc in range(CT):
                        o_ps = pmm.tile([128, CHW], F32, tag="mm")
                        nc.tensor.matmul(o_ps[:], wo_r[0][:, oc * 128:(oc + 1) * 128], yT[0][:],
                                         start=True, stop=False)
                        nc.tensor.matmul(o_ps[:], wo_r[1][:, oc * 128:(oc + 1) * 128], yT[1][:],
                                         start=False, stop=True)
                        o_sb = obp.tile([128, CHW], F16, tag="osb")
                        if oc % 2 == 0:
                            nc.scalar.copy(o_sb[:], o_ps[:])
                        else:
                            nc.vector.tensor_copy(o_sb[:], o_ps[:])
                        nc.sync.dma_start(partial[oc * 128:(oc + 1) * 128, :], o_sb[:])
                    rs_out = drp.tile([2 * D, CHW], F16, tag="rsout", name=f"rsout{g}")
                    nc.gpsimd.collective_compute(
                        "ReduceScatter", mybir.AluOpType.add,
                        replica_groups=[list(range(NC))],
                        ins=[partial.opt()], outs=[rs_out.opt()],
                    )
                    nc.sync.dma_start(outp_d[:, g * CHW:(g + 1) * CHW], rs_out[:])

    nc.compile()
    return nc


def host_prepare(x, conv_w, w_attn, w_proj):
    """Build per-core input maps (fp16 wire format)."""
    xf = x.reshape(B * T, C)                           # token-major
    convw = np.ascontiguousarray(conv_w[:, 0, :]).astype(np.float32)

    t = np.arange(T, dtype=np.float64)
    inv_freq = 1.0 / (10000.0 ** (np.arange(0, D, 2, dtype=np.float64) / D))
    freqs = np.outer(inv_freq, t)                      # [64, T]
    cos64 = np.cos(freqs).astype(np.float32)
    sin64 = np.sin(freqs).astype(np.float32)

    masks = np.zeros((STW, 4 * CHW), dtype=np.float16)
    s_idx = np.arange(STW)[:, None]
    t_idx = np.arange(CHW)[None, :]
    for i in range(4):
        masks[:, i * CHW:(i + 1) * CHW] = (i * STW + s_idx <= t_idx).astype(np.float16)

    ones = np.ones((128, 128), dtype=np.float16)
    ident = np.eye(128, dtype=np.float16)

    scale = 1.0 / np.sqrt(np.float32(D))
    in_maps = []
    for c in range(NC):
        tok0 = c * CHW
        xsh = np.zeros((C, XW), dtype=np.float16)
        xsh[:, 2:] = xf[tok0:tok0 + CHW].T.astype(np.float16)
        if c % NCH != 0:                               # halo from previous chunk
            xsh[:, 0:2] = xf[tok0 - 2:tok0].T.astype(np.float16)
        h0 = c * HPC
        rq = slice(h0 * D, (h0 + HPC) * D)
        wq_c = np.ascontiguousarray((w_attn[rq, :] * scale).T).astype(np.float16)
        wk_c = np.ascontiguousarray(w_attn[C + rq.start:C + rq.stop, :].T).astype(np.float16)
        wv_c = np.ascontiguousarray(w_attn[2 * C + rq.start:2 * C + rq.stop, :].T).astype(np.float16)
        wo_c = np.ascontiguousarray(w_proj[:, rq].T).astype(np.float16)
        in_maps.append({
            "xsh": xsh, "wq": wq_c, "wk": wk_c, "wv": wv_c, "wo": wo_c,
            "cos64": cos64, "sin64": sin64, "masks": masks, "convw": convw,
            "ones": ones, "ident": ident,
        })
    return in_maps


def host_finish(results):
    outT = np.concatenate([r["outp"] for r in results], axis=0)   # [C, B*T] fp16
    return outT.astype(np.float32).reshape(C, B, T).transpose(1, 2, 0)


_CACHE = {}


def kernel(x, conv_w, w_attn, w_proj):
    x = np.ascontiguousarray(x, dtype=np.float32)
    conv_w = np.ascontiguousarray(conv_w, dtype=np.float32)
    w_attn = np.ascontiguousarray(w_attn, dtype=np.float32)
    w_proj = np.ascontiguousarray(w_proj, dtype=np.float32)
    if "nc" not in _CACHE:
        _CACHE["nc"] = build_program()
    in_maps = host_prepare(x, conv_w, w_attn, w_proj)
    res = bass_utils.run_bass_kernel_spmd(_CACHE["nc"], in_maps, core_ids=list(range(NC)))
    return host_finish(res.results)


# revision 10
# speedup vs baseline: 1.4747x; 1.0294x over previous
"""CausalSelfAttention (depthwise-conv + RoPE + causal SDPA + proj) on 8 Trainium2 cores.

v3: minimize per-call host<->device traffic (the dominant cost through this
dispatch path): few consolidated fp16 arg tensors, sequence-sharded x with a
device-side AllGather, head-tensor-parallel fp16 compute (fp32 PSUM
accumulate), per-chunk ReduceScatter of the output-projection partials, and
on-device generation of identity/ones/causal masks (affine_select).

Per-core args: xsh [2048,514] f16 (512-token chunk, transposed, 2-col conv
halo), qkv [6144,256] f16 (this core's 2 heads' QKV weight columns, q
pre-scaled), wb [384,2048] f16 (proj rows + RoPE cos/sin tables), convw
[2048,3] f32. Output: outp [256,4096] f16 (this core's 256 output channels of
the summed projection, all 4096 tokens)."""
import sys
sys.path.insert(0, '/opt/trn_rl_repo')
import numpy as np
import concourse.bass as bass
import concourse.mybir as mybir
import concourse.tile as tile
from concourse import bacc
from concourse import bass_utils
from concourse import masks as bmasks

F32 = mybir.dt.float32
F16 = mybir.dt.float16
AF = mybir.ActivationFunctionType
OP = mybir.AluOpType

B, T, C = 2, 2048, 2048
H, D = 16, 128
NC = 8
HPC = H // NC          # heads per core = 2
CHW = 512              # chunk width (tokens)
NCH = T // CHW         # chunks per batch = 4
TCH = B * NCH          # total chunks = 8
CT = C // 128          # 16 c-tiles
STW = 128              # s-tile width
XW = CHW + 2           # shipped chunk width incl. 2-col conv halo


def build_program():
    nc = bacc.Bacc("TRN2", target_bir_lowering=False, debug=False, num_devices=NC)

    xsh_d = nc.dram_tensor("xsh", [C, XW], F16, kind="ExternalInput").ap()
    qkv_d = nc.dram_tensor("qkv", [3 * C, HPC * D], F16, kind="ExternalInput").ap()
    wb_d = nc.dram_tensor("wb", [HPC * D + 128, C], F16, kind="ExternalInput").ap()
    convw_d = nc.dram_tensor("convw", [C, 3], F32, kind="ExternalInput").ap()
    outp_d = nc.dram_tensor("outp", [HPC * D, B * T], F16, kind="ExternalOutput").ap()

    from contextlib import ExitStack
    with tile.TileContext(nc) as tc:
        with ExitStack() as stack:
            dram = stack.enter_context(tc.tile_pool(name="dram", bufs=1, space="DRAM"))
            drp = stack.enter_context(tc.tile_pool(name="drp", bufs=2, space="DRAM"))
            wr = stack.enter_context(tc.tile_pool(name="wr", bufs=1))
            cst = stack.enter_context(tc.tile_pool(name="const", bufs=1))
            xtp = stack.enter_context(tc.tile_pool(name="xt", bufs=2))
            xcp = stack.enter_context(tc.tile_pool(name="xc", bufs=1))
            tmp = stack.enter_context(tc.tile_pool(name="tmp", bufs=2))
            rpp = stack.enter_context(tc.tile_pool(name="rp", bufs=2))
            rdp = stack.enter_context(tc.tile_pool(name="rd", bufs=2))
            qsp = stack.enter_context(tc.tile_pool(name="qs", bufs=1))
            kvp = stack.enter_context(tc.tile_pool(name="kv", bufs=1))
            vstgp = stack.enter_context(tc.tile_pool(name="vstg", bufs=2))
            ep = stack.enter_context(tc.tile_pool(name="e", bufs=3))
            yp = stack.enter_context(tc.tile_pool(name="y", bufs=1))
            obp = stack.enter_context(tc.tile_pool(name="ob", bufs=4))
            pmm = stack.enter_context(tc.tile_pool(name="pmm", bufs=2, space="PSUM"))
            pS = stack.enter_context(tc.tile_pool(name="pS", bufs=2, space="PSUM"))
            pU = stack.enter_context(tc.tile_pool(name="pU", bufs=2, space="PSUM"))
            pD = stack.enter_context(tc.tile_pool(name="pD", bufs=2, space="PSUM"))
            # ---- AllGather x shards: every core gets the full sequence ----
            ag_in = dram.tile([C, XW], F16, name="ag_in")
            nc.gpsimd.dma_start(ag_in[:], xsh_d[:])
            xg = dram.tile([NC * C, XW], F16, name="xg")
            nc.gpsimd.collective_compute(
                "AllGather", mybir.AluOpType.bypass,
                replica_groups=[list(range(NC))],
                ins=[ag_in.opt()], outs=[xg.opt()],
            )

            # ---- constants (generated or unpacked from wb) ----
            c16s = tmp.tile([64, T], F16, tag="tb16", name="c16s")
            nc.sync.dma_start(c16s[:], wb_d[HPC * D:HPC * D + 64, :])
            cos32 = cst.tile([64, T], F32, tag="cos32")
            nc.vector.tensor_copy(cos32[:], c16s[:])
            s16s = tmp.tile([64, T], F16, tag="tb16", name="s16s")
            nc.sync.dma_start(s16s[:], wb_d[HPC * D + 64:HPC * D + 128, :])
            sin32 = cst.tile([64, T], F32, tag="sin32")
            nc.vector.tensor_copy(sin32[:], s16s[:])
            ident = cst.tile([128, 128], F16, tag="ident")
            bmasks.make_identity(nc, ident[:])
            ones16 = cst.tile([128, 128], F16, tag="ones16")
            nc.gpsimd.memset(ones16[:], 1.0)
            cw = []
            for ct in range(CT):
                t_ = cst.tile([128, 3], F32, tag=f"cw{ct}")
                nc.sync.dma_start(t_[:], convw_d[ct * 128:(ct + 1) * 128, :])
                cw.append(t_)

            # ---- weights resident in SBUF (fp16, used directly by PE) ----
            wq_r, wk_r, wv_r = [], [], []
            for qi, dst in ((0, wq_r), (1, wk_r), (2, wv_r)):
                for ct in range(CT):
                    t_ = wr.tile([128, HPC * D], F16, tag=f"w{qi}r{ct}")
                    nc.sync.dma_start(t_[:], qkv_d[qi * C + ct * 128:qi * C + (ct + 1) * 128, :])
                    dst.append(t_)
            wo_r = []
            for hi in range(HPC):
                t_ = wr.tile([128, C], F16, tag=f"wor{hi}")
                nc.sync.dma_start(t_[:], wb_d[hi * 128:(hi + 1) * 128, :])
                wo_r.append(t_)

            # ---- main loop over the 8 chunks (b major, ch minor) ----
            for b in range(B):
                k_all = [kvp.tile([D, T], F16, tag=f"k{h}", name=f"kall{b}_{h}") for h in range(HPC)]
                v_all = [kvp.tile([128, T], F16, tag=f"v{h}", name=f"vall{b}_{h}") for h in range(HPC)]
                for ch in range(NCH):
                    g = b * NCH + ch               # global chunk id / xg block
                    t0 = ch * CHW                  # within-batch t offset
                    # ---- load + depthwise causal conv ----
                    xc = []
                    for ct in range(CT):
                        xt = xtp.tile([128, XW], F16, tag="xt")
                        nc.sync.dma_start(xt[:], xg[g * C + ct * 128:g * C + (ct + 1) * 128, :])
                        ta = tmp.tile([128, CHW], F16, tag="t1")
                        nc.scalar.mul(ta[:], xt[:, 0:CHW], cw[ct][:, 0:1])
                        tb = tmp.tile([128, CHW], F16, tag="t2")
                        nc.vector.scalar_tensor_tensor(tb[:], xt[:, 1:CHW + 1], cw[ct][:, 1:2], ta[:], OP.mult, OP.add)
                        xct = xcp.tile([128, CHW], F16, tag=f"xc{ct}")
                        nc.vector.scalar_tensor_tensor(xct[:], xt[:, 2:CHW + 2], cw[ct][:, 2:3], tb[:], OP.mult, OP.add)
                        xc.append(xct)

                    # ---- QKV + rope ----
                    q_sb = []
                    for h in range(HPC):
                        hs = slice(h * D, (h + 1) * D)
                        cs = slice(t0, t0 + CHW)
                        # q
                        q_ps = pmm.tile([128, CHW], F32, tag="mm")
                        for ct in range(CT):
                            nc.tensor.matmul(q_ps[:], wq_r[ct][:, hs], xc[ct][:],
                                             start=(ct == 0), stop=(ct == CT - 1))
                        qt = qsp.tile([128, CHW], F16, tag=f"q{h}")
                        at = rpp.tile([64, CHW], F16, tag="ra")
                        nc.vector.tensor_tensor(at[:], q_ps[0:64, :], cos32[:, cs], OP.mult)
                        mt = rpp.tile([64, CHW], F16, tag="rm")
                        nc.vector.tensor_tensor(mt[:], q_ps[64:128, :], sin32[:, cs], OP.mult)
                        nc.vector.tensor_tensor(qt[0:64, :], at[:], mt[:], OP.subtract)
                        ab = rpp.tile([64, CHW], F16, tag="rb")
                        nc.vector.tensor_tensor(ab[:], q_ps[64:128, :], cos32[:, cs], OP.mult)
                        mb = rpp.tile([64, CHW], F16, tag="rn")
                        nc.vector.tensor_tensor(mb[:], q_ps[0:64, :], sin32[:, cs], OP.mult)
                        nc.vector.tensor_tensor(qt[64:128, :], ab[:], mb[:], OP.add)
                        q_sb.append(qt)
                        # k
                        k_ps = pmm.tile([128, CHW], F32, tag="mm")
                        for ct in range(CT):
                            nc.tensor.matmul(k_ps[:], wk_r[ct][:, hs], xc[ct][:],
                                             start=(ct == 0), stop=(ct == CT - 1))
                        at2 = rpp.tile([64, CHW], F16, tag="ra")
                        nc.vector.tensor_tensor(at2[:], k_ps[0:64, :], cos32[:, cs], OP.mult)
                        mt2 = rpp.tile([64, CHW], F16, tag="rm")
                        nc.vector.tensor_tensor(mt2[:], k_ps[64:128, :], sin32[:, cs], OP.mult)
                        nc.vector.tensor_tensor(k_all[h][0:64, cs], at2[:], mt2[:], OP.subtract)
                        ab2 = rpp.tile([64, CHW], F16, tag="rb")
                        nc.vector.tensor_tensor(ab2[:], k_ps[64:128, :], cos32[:, cs], OP.mult)
                        mb2 = rpp.tile([64, CHW], F16, tag="rn")
                        nc.vector.tensor_tensor(mb2[:], k_ps[0:64, :], sin32[:, cs], OP.mult)
                        nc.vector.tensor_tensor(k_all[h][64:128, cs], ab2[:], mb2[:], OP.add)
                        # v
                        v_ps = pmm.tile([128, CHW], F32, tag="mm")
                        for ct in range(CT):
                            nc.tensor.matmul(v_ps[:], wv_r[ct][:, hs], xc[ct][:],
                                             start=(ct == 0), stop=(ct == CT - 1))
                        vstg = vstgp.tile([128, CHW], F16, tag="vstg")
                        nc.scalar.copy(vstg[:], v_ps[:])
                        for j in range(CHW // 128):
                            tp = pS.tile([128, 128], F16, tag="S")
                            nc.tensor.transpose(tp[:], vstg[:, j * 128:(j + 1) * 128], ident[:])
                            srow = t0 + j * 128
                            nc.vector.tensor_copy(v_all[h][:, srow:srow + 128], tp[:])

                    # ---- attention ----
                    yT = []
                    n_st = 4 * ch + 4
                    for h in range(HPC):
                        U_ps = pU.tile([128, CHW], F32, tag="U")
                        D_ps = pD.tile([128, CHW], F32, tag="Dn")
                        for st in range(n_st):
                            s_ps = pS.tile([128, CHW], F32, tag="S")
                            nc.tensor.matmul(s_ps[:], k_all[h][:, st * STW:(st + 1) * STW], q_sb[h][:],
                                             start=True, stop=True)
                            e = ep.tile([128, CHW], F16, tag="e")
                            nc.scalar.activation(e[:], s_ps[:], AF.Exp)
                            if st >= 4 * ch:
                                i = st - 4 * ch
                                # keep e[s, t] where t - s - i*128 >= 0, else 0
                                nc.gpsimd.affine_select(
                                    out=e[:], in_=e[:],
                                    compare_op=OP.is_ge, fill=0.0,
                                    base=-(i * STW), channel_multiplier=-1,
                                    pattern=[[1, CHW]],
                                )
                            nc.tensor.matmul(U_ps[:], v_all[h][:, st * STW:(st + 1) * STW], e[:],
                                             start=(st == 0), stop=(st == n_st - 1))
                            nc.tensor.matmul(D_ps[:], ones16[:], e[:],
                                             start=(st == 0), stop=(st == n_st - 1))
                        rD = rdp.tile([128, CHW], F32, tag="rd")
                        nc.vector.reciprocal(rD[:], D_ps[:])
                        yt = yp.tile([128, CHW], F16, tag=f"y{h}")
                        nc.vector.tensor_tensor(yt[:], U_ps[:], rD[:], OP.mult)
                        yT.append(yt)

                    # ---- partial proj for this chunk -> DRAM -> ReduceScatter ----
                    partial = drp.tile([C, CHW], F16, tag="part", name=f"part{g}")
                    for oc in range(CT):
                        o_ps = pmm.tile([128, CHW], F32, tag="mm")
                        nc.tensor.matmul(o_ps[:], wo_r[0][:, oc * 128:(oc + 1) * 128], yT[0][:],
                                         start=True, stop=False)
                        nc.tensor.matmul(o_ps[:], wo_r[1][:, oc * 128:(oc + 1) * 128], yT[1][:],
                                         start=False, stop=True)
                        o_sb = obp.tile([128, CHW], F16, tag="osb")
                        if oc % 2 == 0:
                            nc.scalar.copy(o_sb[:], o_ps[:])
                        else:
                            nc.vector.tensor_copy(o_sb[:], o_ps[:])
                        nc.sync.dma_start(partial[oc * 128:(oc + 1) * 128, :], o_sb[:])
                    rs_out = drp.tile([HPC * D, CHW], F16, tag="rsout", name=f"rsout{g}")
                    nc.gpsimd.collective_compute(
                        "ReduceScatter", mybir.AluOpType.add,
                        replica_groups=[list(range(NC))],
                        ins=[partial.opt()], outs=[rs_out.opt()],
                    )
                    nc.sync.dma_start(outp_d[:, g * CHW:(g + 1) * CHW], rs_out[:])

    nc.compile()
    return nc


def host_prepare(x, conv_w, w_attn, w_proj):
    """Build per-core input maps (fp16 wire format, consolidated tensors)."""
    xf = x.reshape(B * T, C)                           # token-major
    convw = np.ascontiguousarray(conv_w[:, 0, :]).astype(np.float32)

    t = np.arange(T, dtype=np.float64)
    inv_freq = 1.0 / (10000.0 ** (np.arange(0, D, 2, dtype=np.float64) / D))
    freqs = np.outer(inv_freq, t)                      # [64, T]
    cos16 = np.cos(freqs).astype(np.float16)
    sin16 = np.sin(freqs).astype(np.float16)

    scale = 1.0 / np.sqrt(np.float32(D))
    in_maps = []
    for c in range(NC):
        tok0 = c * CHW
        xsh = np.zeros((C, XW), dtype=np.float16)
        xsh[:, 2:] = xf[tok0:tok0 + CHW].T.astype(np.float16)
        if c % NCH != 0:                               # halo from previous chunk
            xsh[:, 0:2] = xf[tok0 - 2:tok0].T.astype(np.float16)
        h0 = c * HPC
        rq = slice(h0 * D, (h0 + HPC) * D)
        qkv = np.empty((3 * C, HPC * D), dtype=np.float16)
        qkv[0:C] = (w_attn[rq, :] * scale).T.astype(np.float16)
        qkv[C:2 * C] = w_attn[C + rq.start:C + rq.stop, :].T.astype(np.float16)
        qkv[2 * C:3 * C] = w_attn[2 * C + rq.start:2 * C + rq.stop, :].T.astype(np.float16)
        wb = np.empty((HPC * D + 128, C), dtype=np.float16)
        wb[0:HPC * D] = w_proj[:, rq].T.astype(np.float16)
        wb[HPC * D:HPC * D + 64] = cos16
        wb[HPC * D + 64:HPC * D + 128] = sin16
        in_maps.append({"xsh": xsh, "qkv": qkv, "wb": wb, "convw": convw})
    return in_maps


def host_finish(results):
    outT = np.concatenate([r["outp"] for r in results], axis=0)   # [C, B*T] fp16
    return outT.astype(np.float32).reshape(C, B, T).transpose(1, 2, 0)


_CACHE = {}


def kernel(x, conv_w, w_attn, w_proj):
    x = np.ascontiguousarray(x, dtype=np.float32)
    conv_w = np.ascontiguousarray(conv_w, dtype=np.float32)
    w_attn = np.ascontiguousarray(w_attn, dtype=np.float32)
    w_proj = np.ascontiguousarray(w_proj, dtype=np.float32)
    if "nc" not in _CACHE:
        _CACHE["nc"] = build_program()
    in_maps = host_prepare(x, conv_w, w_attn, w_proj)
    res = bass_utils.run_bass_kernel_spmd(_CACHE["nc"], in_maps, core_ids=list(range(NC)))
    return host_finish(res.results)
